# revision 2
# baseline (speedup 1.0000x reference)
"""AlignmentContrastiveLoss (MrSw) on 8 Trainium2 NeuronCores.

Strategy
--------
align[i,j,r,w] = <im[i,r,:], s[j,w,:]>  with padded regions/words zeroed.
Zeroing the padded rows of `im` and padded words of `s` on the host makes
the matmul output exactly equal to the reference's masked_fill(0) tensor,
so no on-device masking is needed.

Sharding: image batch axis i across 8 cores (16 images/core); s replicated.

Per core, for each word index w (37 of them), the TensorEngine computes
    psum_w[j, (i,r)] = sum_d s[j, w, d] * im[i, r, d]        [128 x 784]
as 8 accumulating K=128 matmuls (stationary = s[:, w, :]^T chunk, moving =
im^T chunk).  Both MrSw reductions then happen in cheap directions:
  - max over w  : running elementwise tensor_max across the 37 psum tiles
  - max over r  : free-dim segmented reduce ([128,16,49] -> [128,16]) per w
  - sum over r  : free-dim segmented reduce of the running max
  - sum over w  : free-dim reduce of the stacked per-w maxes
Output per core is [128 j, 16 i_local] fp32; host transposes and stacks.
"""

import numpy as np
import ml_dtypes

import concourse.bacc as bacc
import concourse.mybir as mybir
import concourse.tile as tile
from concourse.bass_utils import run_bass_kernel_spmd

B = 128          # batch (images == sentences)
L_IM, L_S, D = 50, 40, 1024
R = L_IM - 1     # 49 regions
W = L_S - 3      # 37 words
NCORES = 8
IPC = B // NCORES            # 16 images per core
N = IPC * R                  # 784 = moving free dim
KC = D // 128                # 8 contraction chunks
BF16 = mybir.dt.bfloat16
F32 = mybir.dt.float32

_NC_CACHE = None


def _build():
    nc = bacc.Bacc("TRN2", target_bir_lowering=False, debug=False,
                   num_devices=NCORES)
    # sT layout: [dk(128 partitions), w, k, j]  (row-contiguous per partition)
    sT = nc.dram_tensor("sT", [128, W * KC * B], BF16, kind="ExternalInput")
    # imT layout: [dk(128), k, (i,r)]
    imT = nc.dram_tensor("imT", [128, KC * N], BF16, kind="ExternalInput")
    out = nc.dram_tensor("out", [B, IPC], F32, kind="ExternalOutput")

    with tile.TileContext(nc) as tc:
        with (
            tc.tile_pool(name="persist", bufs=1) as persist,
            tc.tile_pool(name="sw", bufs=4) as swpool,
            tc.tile_pool(name="ps", bufs=4, space="PSUM") as pspool,
        ):
            # Separate per-k tiles so the first matmul only waits on chunk 0;
            # spread issue across three engine queues to parallelize the
            # ~0.6us/issue SWDGE latency during startup.
            issue_engines = [nc.scalar, nc.gpsimd, nc.sync]
            imt = []
            for k in range(KC):
                t = persist.tile([128, N], BF16, tag=f"imt{k}")
                issue_engines[k % len(issue_engines)].dma_start(
                    t[:], imT.ap()[:, k * N:(k + 1) * N])
                imt.append(t)

            runmax = persist.tile([128, N], F32)       # max over w so far
            maxr = persist.tile([128, IPC, W], F32)    # per-w max over r

            for w in range(W):
                s_w = swpool.tile([128, KC * 128], BF16)
                (nc.sync if w else nc.scalar).dma_start(
                    s_w[:], sT.ap()[:, w * KC * B:(w + 1) * KC * B])

                ps = pspool.tile([128, N], F32)
                for k in range(KC):
                    lhsT = s_w[:, k * 128:(k + 1) * 128]
                    nc.tensor.matmul(ps[:, 0:512],
                                     lhsT, imt[k][:, 0:512],
                                     start=(k == 0), stop=(k == KC - 1))
                    nc.tensor.matmul(ps[:, 512:N],
                                     lhsT, imt[k][:, 512:N],
                                     start=(k == 0), stop=(k == KC - 1))

                if w == 0:
                    nc.vector.tensor_copy(runmax[:], ps[:])
                else:
                    nc.vector.tensor_max(runmax[:], runmax[:], ps[:])
                nc.vector.reduce_max(
                    maxr[:, :, w],
                    ps[:].rearrange("p (i r) -> p i r", i=IPC),
                    axis=mybir.AxisListType.X)

            term2 = persist.tile([128, IPC], F32)
            nc.vector.reduce_sum(
                term2[:],
                runmax[:].rearrange("p (i r) -> p i r", i=IPC),
                axis=mybir.AxisListType.X)
            term1 = persist.tile([128, IPC], F32)
            nc.vector.reduce_sum(term1[:], maxr[:], axis=mybir.AxisListType.X)

            res = persist.tile([128, IPC], F32)
            nc.vector.tensor_add(res[:], term1[:], term2[:])
            nc.sync.dma_start(out.ap()[:], res[:])

    nc.compile()
    return nc


def _get_nc():
    global _NC_CACHE
    if _NC_CACHE is None:
        _NC_CACHE = _build()
    return _NC_CACHE


def kernel(im_set, s_seq, im_len, s_len):
    im_set = np.asarray(im_set, dtype=np.float32)
    s_seq = np.asarray(s_seq, dtype=np.float32)
    im_len = np.asarray(im_len).astype(np.int64)
    s_len = np.asarray(s_len).astype(np.int64)

    im = im_set[:, 1:, :].copy()          # [B, R, D]
    s = s_seq[:, 1:-2, :].copy()          # [B, W, D]
    il = im_len - 1
    sl = s_len - 3
    im *= (np.arange(R)[None, :] < il[:, None])[:, :, None]
    s *= (np.arange(W)[None, :] < sl[:, None])[:, :, None]

    # sT[dk, w, k, j] = s[j, w, k*128+dk]
    sT = (s.transpose(2, 1, 0)                 # [D, W, B]
          .reshape(KC, 128, W, B)              # [k, dk, w, j]
          .transpose(1, 2, 0, 3)               # [dk, w, k, j]
          .reshape(128, W * KC * B)
          .astype(ml_dtypes.bfloat16))

    in_maps = []
    for c in range(NCORES):
        im_c = im[c * IPC:(c + 1) * IPC]       # [IPC, R, D]
        imT = (im_c.reshape(N, D)
               .T                              # [D, N]
               .reshape(KC, 128, N)            # [k, dk, ir]
               .transpose(1, 0, 2)             # [dk, k, ir]
               .reshape(128, KC * N)
               .astype(ml_dtypes.bfloat16))
        in_maps.append({"sT": sT, "imT": np.ascontiguousarray(imT)})

    nc = _get_nc()
    res = run_bass_kernel_spmd(nc, in_maps, core_ids=list(range(NCORES)))

    full = np.empty((B, B), dtype=np.float32)
    for c in range(NCORES):
        full[c * IPC:(c + 1) * IPC, :] = res.results[c]["out"].T
    return full


# revision 6
# speedup vs baseline: 1.0263x; 1.0263x over previous
"""AlignmentContrastiveLoss (MrSw) on 8 Trainium2 NeuronCores.

Strategy
--------
align[i,j,r,w] = <im[i,r,:], s[j,w,:]>  with padded regions/words zeroed.
Zeroing the padded rows of `im` and padded words of `s` on the host makes
the matmul output exactly equal to the reference's masked_fill(0) tensor,
so no on-device masking is needed.

Sharding: image batch axis i across 8 cores (16 images/core); s replicated.

Per core, for each word index w (37 of them), the TensorEngine computes
    psum_w[j, (i,r)] = sum_d s[j, w, d] * im[i, r, d]        [128 x 784]
as 8 accumulating K=128 matmuls (stationary = s[:, w, :]^T chunk, moving =
im^T chunk).  Both MrSw reductions then happen in cheap directions:
  - max over w  : running elementwise tensor_max across the 37 psum tiles
  - max over r  : free-dim segmented reduce ([128,16,49] -> [128,16]) per w
  - sum over r  : free-dim segmented reduce of the running max
  - sum over w  : free-dim reduce of the stacked per-w maxes
Output per core is [128 j, 16 i_local] fp32; host transposes and stacks.
"""

import numpy as np
import ml_dtypes

import concourse.bacc as bacc
import concourse.mybir as mybir
import concourse.tile as tile
from concourse.bass_utils import run_bass_kernel_spmd

B = 128          # batch (images == sentences)
L_IM, L_S, D = 50, 40, 1024
R = L_IM - 1     # 49 regions
W = L_S - 3      # 37 words
NCORES = 8
IPC = B // NCORES            # 16 images per core
N = IPC * R                  # 784 = moving free dim
KC = D // 128                # 8 contraction chunks
BF16 = mybir.dt.bfloat16
F32 = mybir.dt.float32

_NC_CACHE = None


def _build():
    nc = bacc.Bacc("TRN2", target_bir_lowering=False, debug=False,
                   num_devices=NCORES)
    # sT layout: [dk(128 partitions), w, k, j]  (row-contiguous per partition)
    sT = nc.dram_tensor("sT", [128, W * KC * B], BF16, kind="ExternalInput")
    # imT layout: [dk(128), k, (i,r)]
    imT = nc.dram_tensor("imT", [128, KC * N], BF16, kind="ExternalInput")
    out = nc.dram_tensor("out", [B, IPC], F32, kind="ExternalOutput")

    with tile.TileContext(nc) as tc:
        with (
            tc.tile_pool(name="persist", bufs=1) as persist,
            tc.tile_pool(name="sw", bufs=4) as swpool,
            tc.tile_pool(name="ps", bufs=3, space="PSUM") as pspool,
            tc.tile_pool(name="warm", bufs=1, space="PSUM") as warmpool,
        ):
            # --- startup DMAs -------------------------------------------
            # The first matmul is gated on s_w0 + imt chunk 0; issue those
            # FIRST, each at the head of its own engine queue (the SWDGE
            # issue cost is ~0.6us per dma_start, serialized per engine).
            s_tiles = [None] * W
            s_tiles[0] = swpool.tile([128, KC * 128], BF16, tag="s_w", name="s_w0")
            nc.sync.dma_start(s_tiles[0][:], sT.ap()[:, 0:KC * B])

            imt = [persist.tile([128, N], BF16, tag=f"imt{k}", name=f"imt{k}")
                   for k in range(KC)]
            issue = [nc.scalar, nc.gpsimd]
            for k in range(KC):
                issue[k % len(issue)].dma_start(
                    imt[k][:], imT.ap()[:, k * N:(k + 1) * N])

            # --- PE warm-up ---------------------------------------------
            # HAM unthrottles the PE clock (1.2 -> 2.4 GHz) after ~3.4us of
            # sustained activity.  Burn dummy matmuls on scratch data while
            # the startup DMAs are in flight so the real stream starts warm.
            dummy = persist.tile([128, 128], BF16)
            nc.gpsimd.memset(dummy[:], 0)
            warm = warmpool.tile([128, 128], F32)
            for _ in range(30):
                nc.tensor.matmul(warm[:], dummy[:], dummy[:],
                                 start=True, stop=True)

            runmax = persist.tile([128, N], F32)       # max over w so far
            maxr = persist.tile([128, IPC, W], F32)    # per-w max over r
            term1a = persist.tile([128, IPC], F32)     # partial sum_w maxr

            for w in range(W):
                if w > 0:
                    s_tiles[w] = swpool.tile([128, KC * 128], BF16, tag="s_w",
                                             name=f"s_w{w}")
                    nc.sync.dma_start(
                        s_tiles[w][:], sT.ap()[:, w * KC * B:(w + 1) * KC * B])
                s_w = s_tiles[w]

                ps = pspool.tile([128, N], F32)
                for k in range(KC):
                    lhsT = s_w[:, k * 128:(k + 1) * 128]
                    nc.tensor.matmul(ps[:, 0:512],
                                     lhsT, imt[k][:, 0:512],
                                     start=(k == 0), stop=(k == KC - 1))
                    nc.tensor.matmul(ps[:, 512:N],
                                     lhsT, imt[k][:, 512:N],
                                     start=(k == 0), stop=(k == KC - 1))

                # On the last w, do the maxr/term1 chain BEFORE the runmax
                # update so the serial DVE tail after the final matmul is
                # as short as possible.
                if w == W - 1:
                    nc.vector.reduce_max(
                        maxr[:, :, w],
                        ps[:].rearrange("p (i r) -> p i r", i=IPC),
                        axis=mybir.AxisListType.X)
                    term1 = persist.tile([128, IPC], F32)
                    nc.vector.reduce_sum(term1[:], maxr[:, :, 32:W],
                                         axis=mybir.AxisListType.X)
                    nc.vector.tensor_add(term1[:], term1[:], term1a[:])
                    nc.vector.tensor_max(runmax[:], runmax[:], ps[:])
                    term2 = persist.tile([128, IPC], F32)
                    nc.vector.reduce_sum(
                        term2[:],
                        runmax[:].rearrange("p (i r) -> p i r", i=IPC),
                        axis=mybir.AxisListType.X)
                    res = persist.tile([128, IPC], F32)
                    nc.vector.tensor_add(res[:], term1[:], term2[:])
                    nc.sync.dma_start(out.ap()[:], res[:])
                else:
                    if w == 0:
                        nc.vector.tensor_copy(runmax[:], ps[:])
                    else:
                        nc.vector.tensor_max(runmax[:], runmax[:], ps[:])
                    nc.vector.reduce_max(
                        maxr[:, :, w],
                        ps[:].rearrange("p (i r) -> p i r", i=IPC),
                        axis=mybir.AxisListType.X)
                    if w == 31:
                        # fold w=0..31 of term1 while there is DVE slack
                        nc.vector.reduce_sum(term1a[:], maxr[:, :, 0:32],
                                             axis=mybir.AxisListType.X)

    nc.compile()
    return nc


def _get_nc():
    global _NC_CACHE
    if _NC_CACHE is None:
        _NC_CACHE = _build()
    return _NC_CACHE


def kernel(im_set, s_seq, im_len, s_len):
    im_set = np.asarray(im_set, dtype=np.float32)
    s_seq = np.asarray(s_seq, dtype=np.float32)
    im_len = np.asarray(im_len).astype(np.int64)
    s_len = np.asarray(s_len).astype(np.int64)

    im = im_set[:, 1:, :].copy()          # [B, R, D]
    s = s_seq[:, 1:-2, :].copy()          # [B, W, D]
    il = im_len - 1
    sl = s_len - 3
    im *= (np.arange(R)[None, :] < il[:, None])[:, :, None]
    s *= (np.arange(W)[None, :] < sl[:, None])[:, :, None]

    # sT[dk, w, k, j] = s[j, w, k*128+dk]
    sT = (s.transpose(2, 1, 0)                 # [D, W, B]
          .reshape(KC, 128, W, B)              # [k, dk, w, j]
          .transpose(1, 2, 0, 3)               # [dk, w, k, j]
          .reshape(128, W * KC * B)
          .astype(ml_dtypes.bfloat16))

    in_maps = []
    for c in range(NCORES):
        im_c = im[c * IPC:(c + 1) * IPC]       # [IPC, R, D]
        imT = (im_c.reshape(N, D)
               .T                              # [D, N]
               .reshape(KC, 128, N)            # [k, dk, ir]
               .transpose(1, 0, 2)             # [dk, k, ir]
               .reshape(128, KC * N)
               .astype(ml_dtypes.bfloat16))
        in_maps.append({"sT": sT, "imT": np.ascontiguousarray(imT)})

    nc = _get_nc()
    res = run_bass_kernel_spmd(nc, in_maps, core_ids=list(range(NCORES)))

    full = np.empty((B, B), dtype=np.float32)
    for c in range(NCORES):
        full[c * IPC:(c + 1) * IPC, :] = res.results[c]["out"].T
    return full
